# revision 1
# baseline (speedup 1.0000x reference)
# Trainium2 Bass kernel v2 for nn_Encoder (6-layer conv-attention encoder).
# Sharding: 4 batch groups x 2-way sequence split (cores 2g, 2g+1 own the two
# halves of batch g's sequence); one f32 AllGather of the residual per layer
# boundary (baseline scheme).
# v2 vs baseline:
#  - fp8(e4m3) DoubleRow matmuls for the q/k projections and the attention
#    scores / p@v matmuls (measured end-to-end: adds ~4e-3 rel err).
#    v-proj / Wo / FFN / LN stay bf16 (fp8 there exceeds the error budget).
#  - q/k channel layout regrouped (folded into host weight packing) so rope
#    runs on dense [128, T] tiles and scores use 64-deep DoubleRow contraction.
#  - exp emits fp8 directly with a -ln(4) bias (self-normalizing via the
#    ones-column Z trick), softmax 1/Z broadcast via the sel2 matmul.
#  - bias/affine/mask work elided when inputs make them no-ops.
import sys
sys.path.insert(0, '/opt/trn_rl_repo')
import numpy as np
import ml_dtypes
from contextlib import ExitStack

from concourse import bacc, tile, mybir
import concourse.bass as bass
from concourse.bass_utils import run_bass_kernel_spmd

B, C, T = 4, 512, 1024
F, KW, L, H = 2048, 3, 6, 8
TH, TE = 512, 516
TP4 = T + 4          # x_t cols: global positions [-2, T+2)
QW = 528             # padded slab width for window tensors (16B aligned)
X8W = 1040           # x8 slab width fp8 (16B aligned), data cols 0:1028
XBW = 1028           # xb slab width bf16
NC8 = 8
BF16 = mybir.dt.bfloat16
F32 = mybir.dt.float32
F8 = mybir.dt.float8e4
AF = mybir.ActivationFunctionType
ALU = mybir.AluOpType
DRM = mybir.MatmulPerfMode.DoubleRow
EPS = 1e-4
P = 128
WS = 64.0            # fp8 weight scale for wq8/wk8 (dequant 1/WS on psum copy)
EXPB = -1.3862944    # exp bias: p8 = exp(s)/4 keeps values under fp8e4 max 240
SWAP_MASK = list(range(16, 32)) + list(range(0, 16))
NCH_E = ((0, 512), (512, 4))
NCH_K = ((0, 512), (512, 512))
NCH_V = ((0, 512), (512, 8))
NCH_H = ((0, 512), (512, 2))

_CACHE = {}
TRACE = False
LAST_RESULT = None
PHASE_MARKS = []


def _mark(nc, name):
    PHASE_MARKS.append((name, nc.next_id()))


def _emit(nc, tc, d, flags, n_layers=L, do_gather=True):
    (has_bias, has_aff, mask_ones) = flags
    ctx = ExitStack()

    def pool(name, bufs, space="SBUF"):
        return ctx.enter_context(tc.tile_pool(name=name, bufs=bufs, space=space))

    pers = pool("pers", 1)
    dram = pool("dram", 1, space="DRAM")
    p_psA = pool("psA", 2, space="PSUM")
    p_raw = pool("raw", 4)       # rope inputs / shuffles (bf16)
    p_pt = pool("pt", 6)         # exp outputs, fp8 slab pairs
    p_bc = pool("bc", 2)
    p_xr = pool("xr", 10)        # residual + LN dx tiles f32
    p_ln = pool("ln", 10)        # LN tmps
    p_ht = pool("ht", 3)         # FFN h slab pairs bf16
    p_wq = pool("wq", 2)
    p_wk = pool("wk", 2)
    p_wv = pool("wv", 2)
    p_wo = pool("wo", 2)
    p_w1 = pool("w1", 3)
    p_w2 = pool("w2", 3)
    p_par = pool("par", 2)

    # persistent state
    x_t = [pers.tile([P, TP4], F32, tag=f"x{m}", name=f"x{m}") for m in range(4)]
    xw = [pers.tile([P, TE], F32, tag=f"xw{m}", name=f"xw{m}") for m in range(4)]
    xb = [pers.tile([P, 2 * XBW], BF16, tag=f"xb{k}", name=f"xb{k}") for k in range(2)]
    x8 = [pers.tile([P, 2 * X8W], F8, tag=f"x8{k}", name=f"x8{k}") for k in range(2)]
    x8w = [pers.tile([P, 2 * QW], F8, tag=f"x8w{k}", name=f"x8w{k}") for k in range(2)]
    kr8 = [pers.tile([P, 2 * T], F8, tag=f"kr{g}", name=f"kr{g}") for g in range(2)]
    qr8 = [pers.tile([P, 2 * QW], F8, tag=f"qr{g}", name=f"qr{g}") for g in range(2)]
    vt8 = [pers.tile([P, 2 * QW], F8, tag=f"vt{j}", name=f"vt{j}") for j in range(4)]
    onorm = [pers.tile([P, QW], BF16, tag=f"on{m}", name=f"on{m}") for m in range(4)]
    x1_t = [pers.tile([P, TE], F32, tag=f"x1{m}", name=f"x1{m}") for m in range(4)]
    x1b = [pers.tile([P, 2 * QW], BF16, tag=f"x1b{k}", name=f"x1b{k}") for k in range(2)]
    r2_t = [pers.tile([33, QW], BF16, tag=f"r2{i}", name=f"r2{i}") for i in range(4)]
    pth8 = [pers.tile([P, 64], F8, tag=f"pth{j}", name=f"pth{j}") for j in range(4)]
    cos_k = pers.tile([P, T], BF16, tag="cosk", name="cosk")
    sin_k = pers.tile([P, T], BF16, tag="sink", name="sink")
    cos_q = pers.tile([P, TE], BF16, tag="cosq", name="cosq")
    sin_q = pers.tile([P, TE], BF16, tag="sinq", name="sinq")
    onesm = pers.tile([P, 256], BF16, tag="onesm", name="onesm")
    sel2 = pers.tile([33, 128], BF16, tag="sel2", name="sel2")
    selab = pers.tile([P, 2], F32, tag="selab", name="selab")
    eps_sb = pers.tile([P, 1], F32, tag="eps", name="eps")
    expb_sb = pers.tile([P, 1], F32, tag="expb", name="expb")
    edgem = pers.tile([P, 2], F32, tag="edgem", name="edgem")
    maskx = pers.tile([P, TE], BF16, tag="maskx", name="maskx") if not mask_ones else None
    maskh = pers.tile([P, 514], BF16, tag="maskh", name="maskh") if not mask_ones else None

    dma = nc.sync.dma_start
    loads = [("cos_k_d", cos_k), ("sin_k_d", sin_k), ("cos_q_d", cos_q),
             ("sin_q_d", sin_q), ("sel2_d", sel2), ("ones_d", onesm),
             ("selab_d", selab), ("edgem_d", edgem)]
    if not mask_ones:
        loads += [("maskx_d", maskx), ("maskh_d", maskh)]
    for name, t in loads:
        dma(t[:, :], d[name][:, :])
    for m in range(4):
        dma(x_t[m][:, :], d["x0_d"][m * P:(m + 1) * P, :])
    nc.vector.memset(eps_sb[:, :], EPS)
    nc.vector.memset(expb_sb[:, :], EXPB)
    for i in range(4):
        nc.vector.memset(r2_t[i][:, :], 0.0)
    for j in range(4):
        nc.vector.memset(vt8[j][:, :], 0.0)
        for s in range(2):
            ones_ap = vt8[j][:, s * QW:s * QW + 528].rearrange(
                "p (hh c) -> p hh c", c=66)[:, :, 64:65]
            nc.vector.memset(ones_ap, 1.0)

    def sl(tl, w):
        """[P, 2, w] slab view of a [P, 2*w] tile."""
        return tl[:, :].rearrange("p (two w) -> p two w", two=2)

    mm = nc.tensor.matmul

    # ---- derived copies of the residual stream ---------------------------
    def x_derived():
        """Rebuild xw/x8w (window, gates q) then xb/x8 (full T) from x_t."""
        for m in range(4):
            # window blend: xw = x_t[0:516]*selA + x_t[512:1028]*selB
            tsel = p_xr.tile([P, TE], F32, tag="xr", name="xr")
            nc.vector.tensor_scalar_mul(tsel[:, :], x_t[m][:, 0:TE],
                                        selab[:, 0:1])
            nc.vector.scalar_tensor_tensor(xw[m][:, :], x_t[m][:, TH:TH + TE],
                                           selab[:, 1:2], tsel[:, :],
                                           op0=ALU.mult, op1=ALU.add)
            (nc.gpsimd if m % 2 else nc.vector).tensor_copy(
                sl(x8w[m // 2], QW)[:, m % 2, 0:TE], xw[m][:, :])
        for m in range(4):
            # full-T bf16 (for k/v projections)
            eng = nc.gpsimd if m < 2 else nc.vector
            eng.tensor_copy(sl(xb[m // 2], XBW)[:, m % 2, :],
                            x_t[m][:, 0:XBW])
        for k in range(2):
            (nc.gpsimd if k == 0 else nc.vector).tensor_copy(
                sl(x8[k], X8W)[:, :, 0:XBW], sl(xb[k], XBW)[:, :, :])

    x_derived()

    # ---- layer-norm helper ------------------------------------------------
    def ln(xr_l, ext, par, affcols, wr):
        xrb_l = []
        for m in range(4):
            xrb = p_ln.tile([P, QW], BF16, tag="lnb", name="lnb")
            nc.vector.tensor_copy(xrb[:, 0:ext], xr_l[m][:, 0:ext])
            xrb_l.append(xrb)
        chunks = ((0, 512), (512, ext - 512)) if ext > 512 else ((0, ext),)
        sum_ps = p_psA.tile([P, 1024], F32, tag="psA", name="psA")
        for (o, n) in chunks:
            for kk in range(4):
                mm(sum_ps[:, o:o + n], onesm[:, 0:128], xrb_l[kk][:, o:o + n],
                   start=(kk == 0), stop=(kk == 3))
        sq_l = []
        for m in range(4):
            sq = p_ln.tile([P, QW], BF16, tag="lnb", name="lnb")
            nc.scalar.activation(sq[:, 0:ext], xrb_l[m][:, 0:ext], AF.Square)
            sq_l.append(sq)
        sq_ps = p_psA.tile([P, 1024], F32, tag="psA", name="psA")
        for (o, n) in chunks:
            for kk in range(4):
                mm(sq_ps[:, o:o + n], onesm[:, 128:256], sq_l[kk][:, o:o + n],
                   start=(kk == 0), stop=(kk == 3))
        mean2 = p_ln.tile([P, TE], F32, tag="lnf", name="lnf")
        nc.scalar.activation(mean2[:, 0:ext], sum_ps[:, 0:ext], AF.Square)
        var = p_ln.tile([P, TE], F32, tag="lnf", name="lnf")
        nc.vector.scalar_tensor_tensor(var[:, 0:ext], sq_ps[:, 0:ext], 1.0,
                                       mean2[:, 0:ext], op0=ALU.mult,
                                       op1=ALU.subtract)
        std = p_ln.tile([P, TE], F32, tag="lnf", name="lnf")
        nc.scalar.activation(std[:, 0:ext], var[:, 0:ext], AF.Sqrt,
                             bias=eps_sb[:, 0:1])
        rstd = p_ln.tile([P, TE], F32, tag="lnf", name="lnf")
        nc.vector.reciprocal(rstd[:, 0:ext], std[:, 0:ext])
        for m in range(4):
            dx = p_xr.tile([P, TE], F32, tag="xr", name="xr")
            nc.vector.tensor_add(dx[:, 0:ext], xr_l[m][:, 0:ext],
                                 sum_ps[:, 0:ext])
            wr(m, dx, rstd)

    # ---- per-layer weight loads ------------------------------------------
    def load_weights(li):
        wq = p_wq.tile([P, 2048], F8, tag="wq", name="wq")
        wk = p_wk.tile([P, 2048], F8, tag="wk", name="wk")
        wv = p_wv.tile([P, 2080], BF16, tag="wv", name="wv")
        wo = p_wo.tile([P, 2048], BF16, tag="wo", name="wo")
        par = p_par.tile([P, 52], F32, tag="par", name="par")
        for t, dn in [(wq, "wq_d"), (wk, "wk_d"), (wv, "wv_d"), (wo, "wo_d"),
                      (par, "par_d")]:
            dma(t[:, :], d[dn][li][:, :])
        return wq, wk, wv, wo, par

    def bias_ap(par, col):
        return par[:, col:col + 1] if has_bias else 0.0

    # ======================= the layer loop ================================
    w_next = load_weights(0)
    for li in range(n_layers):
        last = li == n_layers - 1
        wq, wk, wv, wo, par = w_next

        _mark(nc, f"L{li}.qproj")
        # ---- q projection (fp8 DR over window) + rope -> qr8 -------------
        for g in range(2):
            for i in range(2):
                ci = g * 2 + i
                ps = p_psA.tile([P, 1024], F32, tag="psA", name="psA")
                for kk2 in range(2):
                    lhsT = wq[:, (ci * 2 + kk2) * 256:(ci * 2 + kk2 + 1) * 256] \
                        .rearrange("p (two m) -> p two m", two=2)
                    for (o, n) in NCH_E:
                        rhs = sl(x8w[kk2], QW)[:, :, o:o + n]
                        mm(ps[:, o:o + n], lhsT, rhs, start=(kk2 == 0),
                           stop=(kk2 == 1), perf_mode=DRM)
                if i == 0:
                    raw = p_raw.tile([P, TE], BF16, tag="raw", name="raw")
                    nc.scalar.activation(raw[:, :], ps[:, 0:TE], AF.Identity,
                                         bias=bias_ap(par, 0 + ci),
                                         scale=1.0 / WS)
                    sh = p_raw.tile([P, TE], BF16, tag="raw", name="raw")
                    nc.vector.stream_shuffle(sh[:, :], raw[:, :], SWAP_MASK)
                    t1 = p_raw.tile([P, TE], BF16, tag="raw", name="raw")
                    t2 = p_raw.tile([P, TE], BF16, tag="raw", name="raw")
                    nc.vector.tensor_mul(t1[:, :], raw[:, :], cos_q[:, :])
                    nc.vector.tensor_mul(t2[:, :], sh[:, :], sin_q[:, :])
                    nc.vector.tensor_add(qr8[g][:, 0:TE], t1[:, :], t2[:, :])
                else:
                    nc.scalar.activation(qr8[g][:, QW:QW + TE], ps[:, 0:TE],
                                         AF.Identity,
                                         bias=bias_ap(par, 0 + ci),
                                         scale=1.0 / WS)

        _mark(nc, f"L{li}.kproj")
        # ---- k projection (fp8 DR over full T) + rope -> kr8 --------------
        for g in range(2):
            for i in range(2):
                ci = g * 2 + i
                ps = p_psA.tile([P, 1024], F32, tag="psA", name="psA")
                for kk2 in range(2):
                    lhsT = wk[:, (ci * 2 + kk2) * 256:(ci * 2 + kk2 + 1) * 256] \
                        .rearrange("p (two m) -> p two m", two=2)
                    for (o, n) in NCH_K:
                        rhs = sl(x8[kk2], X8W)[:, :, 2 + o:2 + o + n]
                        mm(ps[:, o:o + n], lhsT, rhs, start=(kk2 == 0),
                           stop=(kk2 == 1), perf_mode=DRM)
                if i == 0:
                    raw = p_raw.tile([P, T], BF16, tag="rawT", name="rawT")
                    nc.scalar.activation(raw[:, :], ps[:, 0:T], AF.Identity,
                                         bias=bias_ap(par, 4 + ci),
                                         scale=1.0 / WS)
                    sh = p_raw.tile([P, T], BF16, tag="rawT", name="rawT")
                    nc.vector.stream_shuffle(sh[:, :], raw[:, :], SWAP_MASK)
                    t1 = p_raw.tile([P, T], BF16, tag="rawT", name="rawT")
                    t2 = p_raw.tile([P, T], BF16, tag="rawT", name="rawT")
                    nc.vector.tensor_mul(t1[:, :], raw[:, :], cos_k[:, :])
                    nc.vector.tensor_mul(t2[:, :], sh[:, :], sin_k[:, :])
                    nc.vector.tensor_add(kr8[g][:, 0:T], t1[:, :], t2[:, :])
                else:
                    nc.scalar.activation(kr8[g][:, T:2 * T], ps[:, 0:T],
                                         AF.Identity,
                                         bias=bias_ap(par, 4 + ci),
                                         scale=1.0 / WS)

        _mark(nc, f"L{li}.vproj")
        # ---- v projection (bf16, full T) -> vt8 ---------------------------
        for j in range(8):
            ps = p_psA.tile([P, 1024], F32, tag="psA", name="psA")
            for kk in range(4):
                lhsT = sl(xb[kk // 2], XBW)[:, kk % 2, 2 + j * P:2 + (j + 1) * P]
                for (o, n) in NCH_V:
                    mm(ps[:, o:o + n], lhsT,
                       wv[:, kk * 520 + o:kk * 520 + o + n],
                       start=(kk == 0), stop=(kk == 3))
            src = ps[:, 0:520].rearrange("p (hh c) -> p hh c", c=65)[:, :, 0:64]
            dst = vt8[j // 2][:, (j % 2) * QW:(j % 2) * QW + 528].rearrange(
                "p (hh c) -> p hh c", c=66)[:, :, 0:64]
            nc.scalar.activation(dst, src, AF.Copy)

        # ---- prefetch next layer's weights --------------------------------
        if not last:
            w_next = load_weights(li + 1)

        _mark(nc, f"L{li}.attn")
        # ---- attention ----------------------------------------------------
        with tc.tile_pool(name="psO", bufs=2, space="PSUM") as p_psO:
            for p in range(4):          # head pairs -> onorm[p]
                ops_pair = []
                for sub in range(2):
                    h = 2 * p + sub
                    g, b = h // 4, h % 4
                    o_ps = p_psO.tile([65, 1024], F32, tag="psO", name="psO")
                    for j2 in range(4):
                        pt = p_pt.tile([P, 2 * QW], F8, tag="pt", name="pt")
                        for jj in range(2):
                            j = 2 * j2 + jj
                            sc = p_psA.tile([P, 1024], F32, tag="psA",
                                            name="psA")
                            lhsT = sl(kr8[g], T)[32 * b:32 * (b + 1), :,
                                                 j * P:(j + 1) * P]
                            for (o, n) in NCH_E:
                                rhs = sl(qr8[g], QW)[32 * b:32 * (b + 1), :,
                                                     o:o + n]
                                mm(sc[:, o:o + n], lhsT, rhs, start=True,
                                   stop=True, perf_mode=DRM,
                                   tile_position=(32 * b, 0))
                            nc.scalar.activation(pt[:, jj * QW:jj * QW + TE],
                                                 sc[:, 0:TE], AF.Exp,
                                                 bias=expb_sb[:, 0:1])
                        ptv = sl(pt, QW)
                        lhsT = sl(vt8[j2], QW)[:, :, h * 66:h * 66 + 65]
                        for (o, n) in NCH_E:
                            mm(o_ps[0:65, o:o + n], lhsT, ptv[:, :, o:o + n],
                               start=(j2 == 0), stop=(j2 == 3),
                               perf_mode=DRM, skip_group_check=True)
                    with nc.allow_low_precision(reason="softmax 1/Z in bf16"):
                        nc.vector.reciprocal(
                            r2_t[p][sub * 32:sub * 32 + 1, 0:TE],
                            o_ps[64:65, 0:TE])
                    ops_pair.append(o_ps)
                bc_ps = p_psA.tile([P, 1024], F32, tag="psA", name="psA")
                for (o, n) in NCH_E:
                    mm(bc_ps[:, o:o + n], sel2[:, :], r2_t[p][:, o:o + n],
                       start=True, stop=True)
                bc = p_bc.tile([P, QW], BF16, tag="bc", name="bc")
                nc.vector.tensor_copy(bc[:, 0:TE], bc_ps[:, 0:TE])
                for sub in range(2):
                    oo = onorm[p][sub * 64:(sub + 1) * 64, 0:TE]
                    nc.vector.tensor_mul(oo, ops_pair[sub][0:64, 0:TE],
                                         bc[sub * 64:(sub + 1) * 64, 0:TE])
                    if has_bias:
                        nc.vector.tensor_scalar_add(
                            oo, oo,
                            par[sub * 64:(sub + 1) * 64, 48 + p:49 + p])

        _mark(nc, f"L{li}.wo_ln1")
        # ---- Wo + residual + LN1 -----------------------------------------
        xr_l = []
        for m in range(4):
            ps = p_psA.tile([P, 1024], F32, tag="psA", name="psA")
            for kk in range(4):
                for (o, n) in NCH_E:
                    mm(ps[:, o:o + n],
                       wo[:, kk * 512 + m * P:kk * 512 + (m + 1) * P],
                       onorm[kk][:, o:o + n], start=(kk == 0), stop=(kk == 3))
            xr = p_xr.tile([P, TE], F32, tag="xr", name="xr")
            nc.vector.scalar_tensor_tensor(xr[:, :], ps[:, 0:TE],
                                           bias_ap(par, 8 + m), xw[m][:, :],
                                           op0=ALU.add, op1=ALU.add)
            xr_l.append(xr)

        def wr_ln1(m, dx, rstd):
            nc.vector.tensor_mul(x1_t[m][:, :], dx[:, 0:TE], rstd[:, 0:TE])
            if has_aff:
                nc.scalar.activation(x1_t[m][:, :], x1_t[m][:, :], AF.Identity,
                                     bias=par[:, 36 + m:37 + m],
                                     scale=par[:, 32 + m:33 + m])
            tgt = sl(x1b[m // 2], QW)[:, m % 2, 0:TE]
            if mask_ones:
                nc.vector.tensor_copy(tgt, x1_t[m][:, :])
                base = (m % 2) * QW
                nc.vector.tensor_scalar_mul(
                    x1b[m // 2][:, base:base + 2],
                    x1b[m // 2][:, base:base + 2], edgem[:, 0:1])
                nc.vector.tensor_scalar_mul(
                    x1b[m // 2][:, base + 514:base + 516],
                    x1b[m // 2][:, base + 514:base + 516], edgem[:, 1:2])
            else:
                nc.vector.tensor_mul(tgt, x1_t[m][:, :], maskx[:, :])

        ln(xr_l, TE, par, None, wr_ln1)

        _mark(nc, f"L{li}.ffn")
        # ---- FFN ----------------------------------------------------------
        with tc.tile_pool(name="psY", bufs=4, space="PSUM") as p_psY:
            y_ps = [p_psY.tile([P, 512], F32, tag="psY", name="psY")
                    for m in range(4)]
            for fm in range(16):
                w1t = p_w1.tile([P, 1536], BF16, tag="w1", name="w1")
                dma(w1t[:, :], d["w1_d"][li][:, fm * 1536:(fm + 1) * 1536])
                h_ps = p_psA.tile([P, 1024], F32, tag="psA", name="psA")
                for (o, n) in NCH_H:
                    bidx = 0
                    for kk in range(4):
                        x1v = sl(x1b[kk // 2], QW)[:, kk % 2, :]
                        for dk in range(3):
                            mm(h_ps[:, o:o + n],
                               w1t[:, bidx * 128:(bidx + 1) * 128],
                               x1v[0:P, dk + o:dk + o + n],
                               start=(bidx == 0), stop=(bidx == 11))
                            bidx += 1
                if fm % 2 == 0:
                    ht_pair = p_ht.tile([P, 2 * QW], BF16, tag="htp", name="htp")
                tgt = ht_pair[:, (fm % 2) * QW:(fm % 2) * QW + 514]
                nc.scalar.activation(tgt, h_ps[:, 0:514], AF.Relu,
                                     bias=bias_ap(par, 16 + fm), scale=1.0)
                base = (fm % 2) * QW
                if mask_ones:
                    nc.vector.tensor_scalar_mul(
                        ht_pair[:, base:base + 1],
                        ht_pair[:, base:base + 1], edgem[:, 0:1])
                    nc.vector.tensor_scalar_mul(
                        ht_pair[:, base + 513:base + 514],
                        ht_pair[:, base + 513:base + 514], edgem[:, 1:2])
                else:
                    nc.vector.tensor_mul(tgt, tgt, maskh[:, :])
                w2t = p_w2.tile([P, 1536], BF16, tag="w2", name="w2")
                dma(w2t[:, :], d["w2_d"][li][:, fm * 1536:(fm + 1) * 1536])
                for m in range(4):
                    for dk in range(3):
                        mm(y_ps[m][:, 0:512],
                           w2t[:, (m * 3 + dk) * 128:(m * 3 + dk + 1) * 128],
                           ht_pair[:, (fm % 2) * QW + dk:(fm % 2) * QW + dk + 512],
                           start=(fm == 0 and dk == 0),
                           stop=(fm == 15 and dk == 2), skip_group_check=True)
            xr2_l = []
            for m in range(4):
                xr2 = p_xr.tile([P, TE], F32, tag="xr", name="xr")
                nc.vector.scalar_tensor_tensor(xr2[:, 0:TH], y_ps[m][:, 0:TH],
                                               bias_ap(par, 12 + m),
                                               x1_t[m][:, 2:2 + TH],
                                               op0=ALU.add, op1=ALU.add)
                xr2_l.append(xr2)

            def wr_ln2(m, dx, rstd):
                x2 = x_t[m][:, 2:2 + TH]
                nc.vector.tensor_mul(x2, dx[:, 0:TH], rstd[:, 0:TH])
                if has_aff:
                    nc.scalar.activation(x2, x2, AF.Identity,
                                         bias=par[:, 44 + m:45 + m],
                                         scale=par[:, 40 + m:41 + m])
                if not mask_ones:
                    nc.vector.tensor_mul(x2, x2, maskx[:, 2:2 + TH])

            ln(xr2_l, TH, par, None, wr_ln2)

        _mark(nc, f"L{li}.bound")
        # ---- output / gather ---------------------------------------------
        if last:
            for m in range(4):
                dma(d["out_d"][m * P:(m + 1) * P, :], x_t[m][:, 2:2 + TH])
        elif not do_gather:
            x_derived()
        else:
            bin_ = dram.tile([C, TH], F32, tag=f"bin{li}", name=f"bin{li}")
            bout = dram.tile([2 * C, TH], F32, tag=f"bout{li}", name=f"bout{li}")
            for m in range(4):
                dma(bin_[m * P:(m + 1) * P, :], x_t[m][:, 2:2 + TH])
            nc.gpsimd.collective_compute(
                "AllGather", ALU.bypass,
                replica_groups=[[0, 1], [2, 3], [4, 5], [6, 7]],
                ins=[bin_[:, :].opt()], outs=[bout[:, :].opt()])
            for m in range(4):
                dma(x_t[m][:, 2:2 + TH], bout[m * P:(m + 1) * P, :])
                dma(x_t[m][:, 2 + TH:2 + T], bout[C + m * P:C + (m + 1) * P, :])
            x_derived()

    ctx.close()


def build_program(flags, n_layers=L, do_gather=True):
    nc = bacc.Bacc(target_bir_lowering=False, trn_type="TRN2", num_devices=NC8)
    d = {}
    d["x0_d"] = nc.declare_dram_parameter("x0", [C, TP4], F32, isOutput=False)
    d["cos_k_d"] = nc.declare_dram_parameter("cos_k", [P, T], BF16, isOutput=False)
    d["sin_k_d"] = nc.declare_dram_parameter("sin_k", [P, T], BF16, isOutput=False)
    d["cos_q_d"] = nc.declare_dram_parameter("cos_q", [P, TE], BF16, isOutput=False)
    d["sin_q_d"] = nc.declare_dram_parameter("sin_q", [P, TE], BF16, isOutput=False)
    d["sel2_d"] = nc.declare_dram_parameter("sel2", [33, 128], BF16, isOutput=False)
    d["ones_d"] = nc.declare_dram_parameter("onesmat", [P, 256], BF16, isOutput=False)
    d["selab_d"] = nc.declare_dram_parameter("selab", [P, 2], F32, isOutput=False)
    d["edgem_d"] = nc.declare_dram_parameter("edgem", [P, 2], F32, isOutput=False)
    (has_bias, has_aff, mask_ones) = flags
    if not mask_ones:
        d["maskx_d"] = nc.declare_dram_parameter("maskx", [P, TE], BF16, isOutput=False)
        d["maskh_d"] = nc.declare_dram_parameter("maskh", [P, 514], BF16, isOutput=False)
    for key, shp, dt in [("wq_d", [P, 2048], F8), ("wk_d", [P, 2048], F8),
                         ("wv_d", [P, 2080], BF16), ("wo_d", [P, 2048], BF16),
                         ("w1_d", [P, 16 * 1536], BF16),
                         ("w2_d", [P, 16 * 1536], BF16),
                         ("par_d", [P, 52], F32)]:
        d[key] = [nc.declare_dram_parameter(f"{key[:-2]}{i}", shp, dt,
                                            isOutput=False) for i in range(L)]
    d["out_d"] = nc.declare_dram_parameter("out", [C, TH], F32, isOutput=True)
    with tile.TileContext(nc) as tc:
        _emit(nc, tc, d, flags, n_layers=n_layers, do_gather=do_gather)
    nc.compile()
    return nc


# ======================= host side =======================

def _f8(x):
    return np.ascontiguousarray(
        np.asarray(x, np.float32).astype(ml_dtypes.float8_e4m3))


def _bf(x):
    return np.ascontiguousarray(
        np.asarray(x, np.float32).astype(ml_dtypes.bfloat16))


def _ocols(g, i):
    """output-channel order for q/k psum chunk (g, i): 4 head blocks x 32."""
    m = np.arange(128)
    return (4 * g + m // 32) * 64 + i * 32 + (m % 32)


def _pack_qk8(W):
    """W [C, C] (already scaled). Returns [128, 2048] fp8: blocks of 256 cols
    per (ci=g*2+i, kk2), each [p, ii, mcol] = W[ocol(m), (2kk2+ii)*128+p]."""
    out = np.zeros((128, 2048), np.float32)
    for g in range(2):
        for i in range(2):
            ci = g * 2 + i
            Wc = W[_ocols(g, i), :] * WS          # [128 m, 512 cin]
            for kk2 in range(2):
                blk = Wc[:, kk2 * 256:(kk2 + 1) * 256].reshape(128, 2, 128)
                out[:, (ci * 2 + kk2) * 256:(ci * 2 + kk2 + 1) * 256] = \
                    blk.transpose(2, 1, 0).reshape(128, 256)
    return _f8(out)


def _rope_tables(tvals):
    """cos/sin for the regrouped layout: every partition is a rope channel,
    lane l (mod 32): l<16 -> theta[l], sin=-; l>=16 -> theta[l-16], sin=+."""
    DRP = 32
    theta = 1.0 / (10000.0 ** (np.arange(0, DRP, 2) / DRP))
    cos = np.ones((128, len(tvals)), np.float32)
    sin = np.zeros((128, len(tvals)), np.float32)
    for r in range(128):
        lc = r % 32
        if lc < 16:
            ang = theta[lc] * tvals
            cos[r] = np.cos(ang); sin[r] = -np.sin(ang)
        else:
            ang = theta[lc - 16] * tvals
            cos[r] = np.cos(ang); sin[r] = np.sin(ang)
    return cos, sin


def _pack_weights(inputs):
    per_layer = []
    for li in range(L):
        Wq = np.asarray(inputs['Wq'][li][:, :, 0], np.float32) / 8.0
        Wk = np.asarray(inputs['Wk'][li][:, :, 0], np.float32)
        Wv = np.asarray(inputs['Wv'][li][:, :, 0], np.float32)
        Wo = np.asarray(inputs['Wo'][li][:, :, 0], np.float32)
        W1 = np.asarray(inputs['W1'][li], np.float32)  # [F, C, 3]
        W2 = np.asarray(inputs['W2'][li], np.float32)  # [C, F, 3]

        wq_p = _pack_qk8(Wq)
        wk_p = _pack_qk8(Wk)

        def packT(W):
            WT = W.T
            return np.concatenate([WT[kk * 128:(kk + 1) * 128, :]
                                   for kk in range(4)], axis=1)

        wo_p = _bf(packT(Wo))
        WvT = Wv.T
        wv_p = np.zeros((128, 2080), np.float32)
        for kk in range(4):
            blk = WvT[kk * 128:(kk + 1) * 128, :]
            for hh in range(8):
                wv_p[:, kk * 520 + hh * 65:kk * 520 + hh * 65 + 64] = \
                    blk[:, hh * 64:(hh + 1) * 64]
        w1_p = np.zeros((128, 16 * 1536), np.float32)
        for fm in range(16):
            for kk in range(4):
                for dk in range(3):
                    b = kk * 3 + dk
                    w1_p[:, fm * 1536 + b * 128:fm * 1536 + (b + 1) * 128] = \
                        W1[fm * 128:(fm + 1) * 128,
                           kk * 128:(kk + 1) * 128, dk].T
        w2_p = np.zeros((128, 16 * 1536), np.float32)
        for fm in range(16):
            for m in range(4):
                for dk in range(3):
                    b = m * 3 + dk
                    w2_p[:, fm * 1536 + b * 128:fm * 1536 + (b + 1) * 128] = \
                        W2[m * 128:(m + 1) * 128,
                           fm * 128:(fm + 1) * 128, dk].T
        par = np.zeros((128, 52), np.float32)

        def col4(vec):
            return np.asarray(vec, np.float32).reshape(4, 128).T

        # q/k biases in the regrouped chunk order (cols 0:4 q, 4:8 k)
        bq = np.asarray(inputs['bq'][li], np.float32) / 8.0
        bk = np.asarray(inputs['bk'][li], np.float32)
        for g in range(2):
            for i in range(2):
                ci = g * 2 + i
                par[:, 0 + ci] = bq[_ocols(g, i)]
                par[:, 4 + ci] = bk[_ocols(g, i)]
        par[:, 8:12] = col4(inputs['bo'][li])
        par[:, 12:16] = col4(inputs['c2'][li])
        par[:, 16:32] = np.asarray(inputs['c1'][li], np.float32).reshape(16, 128).T
        par[:, 32:36] = col4(inputs['g1'][li])
        par[:, 36:40] = col4(inputs['be1'][li])
        par[:, 40:44] = col4(inputs['g2'][li])
        par[:, 44:48] = col4(inputs['be2'][li])
        par[:, 48:52] = col4(inputs['bv'][li])
        per_layer.append(dict(wq=wq_p, wk=wk_p, wv=_bf(wv_p), wo=wo_p,
                              w1=_bf(w1_p), w2=_bf(w2_p), par=par))
    return per_layer


def kernel(**inputs):
    inputs = {k: np.asarray(v) for k, v in inputs.items()}
    x = inputs['x'].astype(np.float32) * inputs['x_mask'].astype(np.float32)
    has_bias = bool(any(np.any(np.asarray(inputs[k]) != 0)
                        for k in ('bq', 'bk', 'bv', 'bo', 'c1', 'c2')))
    has_aff = bool(np.any(np.asarray(inputs['g1']) != 1) or
                   np.any(np.asarray(inputs['be1']) != 0) or
                   np.any(np.asarray(inputs['g2']) != 1) or
                   np.any(np.asarray(inputs['be2']) != 0))
    mask_ones = bool(np.all(np.asarray(inputs['x_mask']) == 1))
    flags = (has_bias, has_aff, mask_ones)
    if flags not in _CACHE:
        _CACHE[flags] = build_program(flags)
    nc = _CACHE[flags]

    wl = _pack_weights(inputs)
    cos_k, sin_k = _rope_tables(np.arange(T, dtype=np.float64))
    onesmat = np.concatenate([np.full((128, 128), -1.0 / 512, np.float32),
                              np.full((128, 128), 1.0 / 512, np.float32)],
                             axis=1)
    sel2 = np.zeros((33, 128), np.float32)
    sel2[0, 0:64] = 1.0
    sel2[32, 64:128] = 1.0

    in_maps = []
    for core in range(NC8):
        g, h = core // 2, core % 2
        t0 = h * TH
        xp = np.zeros((C, TP4), np.float32)
        xp[:, 2:2 + T] = x[g]
        cos_q, sin_q = _rope_tables(np.arange(t0 - 2, t0 + 514,
                                              dtype=np.float64))
        im = {
            "x0": xp,
            "cos_k": _bf(cos_k), "sin_k": _bf(sin_k),
            "cos_q": _bf(cos_q), "sin_q": _bf(sin_q),
            "selab": np.repeat(np.array([[1.0 - h, float(h)]], np.float32),
                               128, axis=0),
            "edgem": np.repeat(np.array([[float(h), 1.0 - h]],
                                np.float32), 128, axis=0),
            "sel2": _bf(sel2), "onesmat": _bf(onesmat),
        }
        if not mask_ones:
            mx = np.ones((128, TE), np.float32)
            mh = np.ones((128, 514), np.float32)
            if h == 0:
                mx[:, 0:2] = 0; mh[:, 0:1] = 0
            else:
                mx[:, 514:516] = 0; mh[:, 513:514] = 0
            im["maskx"] = _bf(mx)
            im["maskh"] = _bf(mh)
        for li in range(L):
            w = wl[li]
            im[f"wq{li}"] = w['wq']; im[f"wk{li}"] = w['wk']
            im[f"wv{li}"] = w['wv']; im[f"wo{li}"] = w['wo']
            im[f"w1{li}"] = w['w1']; im[f"w2{li}"] = w['w2']
            im[f"par{li}"] = w['par']
        in_maps.append(im)

    global LAST_RESULT
    res = run_bass_kernel_spmd(nc, in_maps, core_ids=list(range(NC8)),
                               trace=TRACE)
    LAST_RESULT = res
    out = np.zeros((B, C, T), np.float32)
    for g in range(B):
        out[g, :, 0:TH] = res.results[2 * g]["out"]
        out[g, :, TH:T] = res.results[2 * g + 1]["out"]
    out_dt = np.asarray(inputs['x']).dtype
    return out.astype(out_dt)

